# revision 7
# baseline (speedup 1.0000x reference)
"""DBRX MoE experts kernel for Trainium2 (8 NeuronCores).

Strategy:
  - Router (logits -> softmax -> top-2 -> renormalize) computed on host in numpy
    (0.01% of FLOPs); it determines the token->expert dispatch, i.e. the sharding.
  - Tensor-parallel over the FFN intermediate dim across 8 cores: core c owns
    I-slice [c*512:(c+1)*512) of every expert (ws rows for gate and up, w2s cols).
  - Top-2 sparsity: tokens are packed per expert (padded to 256-token blocks);
    each core runs gate/up matmuls (contraction D=2048), SwiGLU, down matmul
    (contraction I_shard=512), scales rows by combine weights, and scatters rows
    into a slot buffer (slot = 2*token + k, k = rank of expert in the token's
    top-2). Pads go to dump slots.
  - Matmuls run in fp32r (11-bit mantissa, full PE rate at free dim >= 256).
    All weight/activation inputs are pre-rounded to fp32r on host (bit-exact
    with the device rounding); the on-chip h = silu(gate)*up write rounds to
    fp32r for free via the DVE output dtype.
  - Slot pairs are combined locally (out[t] = slots[2t] + slots[2t+1]), then a
    ReduceScatter over the 8 cores sums the I-shard partials; core c returns
    token rows [c*512:(c+1)*512). Host concatenates the shards.
"""

import math

import numpy as np

T = 4096
D = 2048
E = 8
I = 4096
TOPK = 2
NCORES = 8
ISH = I // NCORES  # 512, per-core I shard
BLK = 256  # token block (matmul free dim for gate/up)
P = 128
DCH = D // P  # 16 d-chunks
ICH = ISH // P  # 4 i-chunks


def _round_fp32r(x: np.ndarray) -> np.ndarray:
    """Round-to-nearest-even to 11 explicit mantissa bits (device-verified bit-exact)."""
    b = np.ascontiguousarray(x, dtype=np.float32).view(np.uint32).astype(np.uint64)
    bias = ((b >> 12) & 1) + np.uint64(0x7FF)
    r = ((b + bias) >> 12 << 12).astype(np.uint32)
    return r.view(np.float32)


def _host_router(x, router_w):
    """Replicate reference routing in numpy (fp32)."""
    logits = (x.astype(np.float64) @ router_w.astype(np.float64).T).astype(np.float32)
    m = logits.max(axis=-1, keepdims=True)
    ex = np.exp((logits - m).astype(np.float32))
    probs = ex / ex.sum(axis=-1, keepdims=True)
    # top-2, ties to lower index (matches jax.lax.top_k)
    top1 = probs.argmax(axis=-1)
    p = probs.copy()
    p[np.arange(T), top1] = -1.0
    top2 = p.argmax(axis=-1)
    w1 = probs[np.arange(T), top1]
    w2 = probs[np.arange(T), top2]
    s = w1 + w2
    return top1.astype(np.int64), top2.astype(np.int64), (w1 / s).astype(np.float32), (w2 / s).astype(np.float32)


_CACHE: dict = {}


def _build_bass(nblk: list[int], npad: int, nslots: int):
    """Build the 8-core SPMD Bass program. nblk[e] = number of 256-token blocks
    for expert e; npad = total packed (padded) tokens; nslots = slot rows."""
    import concourse.bacc as bacc
    import concourse.bass as bass
    import concourse.mybir as mybir
    import concourse.tile as tile

    f32 = mybir.dt.float32
    f32r = mybir.dt.float32r
    nsub = npad // P  # 128-row subblocks

    nc = bacc.Bacc("TRN2", target_bir_lowering=False)

    nblk_tot = npad // BLK
    xtp_d = nc.dram_tensor("xtp", [P, nblk_tot, DCH, BLK], f32r, kind="ExternalInput")
    wst_d = nc.dram_tensor("wst", [E, DCH, P, 2 * ISH], f32r, kind="ExternalInput")
    w2st_d = nc.dram_tensor("w2st", [E, ICH, P, D], f32r, kind="ExternalInput")
    cw_d = nc.dram_tensor("cw", [P, nsub], f32, kind="ExternalInput")
    sidx_d = nc.dram_tensor("sidx", [P, nsub], mybir.dt.int32, kind="ExternalInput")
    out_d = nc.dram_tensor("out", [T // NCORES, D], f32, kind="ExternalOutput")

    with tile.TileContext(nc) as tc:
        with (
            tc.tile_pool(name="dram", bufs=1, space="DRAM") as dram_pool,
            tc.tile_pool(name="wpool", bufs=18) as wpool,
            tc.tile_pool(name="w2pool", bufs=5) as w2pool,
            tc.tile_pool(name="xpool", bufs=2) as xpool,
            tc.tile_pool(name="spool", bufs=4) as spool,
            tc.tile_pool(name="hpool", bufs=2) as hpool,
            tc.tile_pool(name="opool", bufs=2) as opool,
            tc.tile_pool(name="cpool", bufs=2) as cpool,
            tc.tile_pool(name="const", bufs=1) as const_pool,
            tc.tile_pool(name="ph", bufs=4, space="PSUM") as ph_pool,
            tc.tile_pool(name="po", bufs=4, space="PSUM") as po_pool,
        ):
            slots = dram_pool.tile([nslots, D], f32)
            comb = dram_pool.tile([T, D], f32)
            rs_out = dram_pool.tile([T // NCORES, D], f32)

            # combine weights + scatter indices, resident
            cw_sb = const_pool.tile([P, nsub], f32)
            sidx_sb = const_pool.tile([P, nsub], mybir.dt.int32)
            nc.sync.dma_start(cw_sb[:], cw_d[:])
            nc.sync.dma_start(sidx_sb[:], sidx_d[:])

            gblk = 0
            for e in range(E):
                wst_tiles = []
                for dc in range(DCH):
                    wt = wpool.tile([P, 2 * ISH], f32r, tag="wst")
                    nc.sync.dma_start(wt[:], wst_d[e, dc])
                    wst_tiles.append(wt)
                w2_tiles = []
                for ic in range(ICH):
                    w2t = w2pool.tile([P, D], f32r, tag="w2st")
                    nc.sync.dma_start(w2t[:], w2st_d[e, ic])
                    w2_tiles.append(w2t)

                for _b in range(nblk[e]):
                    xt = xpool.tile([P, DCH, BLK], f32r, tag="xt")
                    nc.sync.dma_start(xt[:], xtp_d[:, gblk])

                    # phase 1: gate/up in ic-pairs; each accumulation group gets
                    # its own PSUM bank (start=True clears the whole bank)
                    hT = hpool.tile([P, ICH, BLK], f32r, tag="hT")
                    for half in range(ICH // 2):
                        phg = [
                            ph_pool.tile([P, BLK], f32, tag="ph", name=f"phg_{gblk}_{half}_{j}")
                            for j in range(2)
                        ]
                        phu = [
                            ph_pool.tile([P, BLK], f32, tag="ph", name=f"phu_{gblk}_{half}_{j}")
                            for j in range(2)
                        ]
                        for dc in range(DCH):
                            wt = wst_tiles[dc]
                            for j in range(2):
                                ic = half * 2 + j
                                nc.tensor.matmul(
                                    phg[j][:],
                                    wt[:, ic * P : (ic + 1) * P],
                                    xt[:, dc, :],
                                    start=(dc == 0),
                                    stop=(dc == DCH - 1),
                                )
                                nc.tensor.matmul(
                                    phu[j][:],
                                    wt[:, ISH + ic * P : ISH + (ic + 1) * P],
                                    xt[:, dc, :],
                                    start=(dc == 0),
                                    stop=(dc == DCH - 1),
                                )
                        for j in range(2):
                            ic = half * 2 + j
                            sg = spool.tile([P, BLK], f32, tag="sg")
                            nc.scalar.activation(
                                sg[:], phg[j][:], mybir.ActivationFunctionType.Silu
                            )
                            nc.vector.tensor_mul(hT[:, ic, :], sg[:], phu[j][:])

                    # phase 3: down proj per 128-token subblock
                    for s in range(BLK // P):
                        gsub = gblk * (BLK // P) + s
                        osb = opool.tile([P, D], f32, tag="osb")
                        for dt_i in range(D // 512):
                            po_t = po_pool.tile([P, 512], f32, tag="po")
                            for ic in range(ICH):
                                nc.tensor.matmul(
                                    po_t[:],
                                    hT[:, ic, s * P : (s + 1) * P],
                                    w2_tiles[ic][:, dt_i * 512 : (dt_i + 1) * 512],
                                    start=(ic == 0),
                                    stop=(ic == ICH - 1),
                                )
                            # evacuate + scale by combine weight (split ACT/DVE)
                            if dt_i < 2:
                                nc.scalar.activation(
                                    osb[:, dt_i * 512 : (dt_i + 1) * 512],
                                    po_t[:],
                                    mybir.ActivationFunctionType.Copy,
                                    scale=cw_sb[:, gsub : gsub + 1],
                                )
                            else:
                                nc.vector.tensor_scalar_mul(
                                    osb[:, dt_i * 512 : (dt_i + 1) * 512],
                                    po_t[:],
                                    cw_sb[:, gsub : gsub + 1],
                                )
                        nc.gpsimd.indirect_dma_start(
                            out=slots[:],
                            out_offset=bass.IndirectOffsetOnAxis(
                                ap=sidx_sb[:, gsub : gsub + 1], axis=0
                            ),
                            in_=osb[:],
                            in_offset=None,
                        )
                    gblk += 1

            # combine slot pairs: comb[t] = slots[2t] + slots[2t+1]
            slots_pair = slots[: 2 * T].rearrange("(t k) d -> t k d", k=2)
            CCH = 512
            for j in range(T // P):
                for q in range(D // CCH):
                    a0 = cpool.tile([P, CCH], f32, tag="c0")
                    a1 = cpool.tile([P, CCH], f32, tag="c1")
                    nc.sync.dma_start(
                        a0[:], slots_pair[j * P : (j + 1) * P, 0, q * CCH : (q + 1) * CCH]
                    )
                    nc.sync.dma_start(
                        a1[:], slots_pair[j * P : (j + 1) * P, 1, q * CCH : (q + 1) * CCH]
                    )
                    cs = cpool.tile([P, CCH], f32, tag="cs")
                    nc.vector.tensor_add(cs[:], a0[:], a1[:])
                    nc.sync.dma_start(
                        comb[j * P : (j + 1) * P, q * CCH : (q + 1) * CCH], cs[:]
                    )

            nc.gpsimd.collective_compute(
                "ReduceScatter",
                mybir.AluOpType.add,
                replica_groups=[list(range(NCORES))],
                ins=[comb[:].opt()],
                outs=[rs_out[:].opt()],
            )
            nc.sync.dma_start(out_d[:], rs_out[:])

    nc.compile()
    return nc


def _prepare(hidden_states, router_w, ws, w2s):
    """Host-side routing, packing, transposes, fp32r rounding. Returns
    (nblk, npad, nslots, shared inputs dict, per-core weight arrays)."""
    x = np.asarray(hidden_states, dtype=np.float32).reshape(T, D)
    router_w = np.asarray(router_w, dtype=np.float32)
    ws = np.asarray(ws, dtype=np.float32)
    w2s = np.asarray(w2s, dtype=np.float32)

    top1, top2, w1, w2 = _host_router(x, router_w)

    # per-expert token lists with slot (2t+k) and weight
    toks: list[list[int]] = [[] for _ in range(E)]
    slots_l: list[list[int]] = [[] for _ in range(E)]
    cws: list[list[float]] = [[] for _ in range(E)]
    for k, (ti, wi) in enumerate([(top1, w1), (top2, w2)]):
        for t in range(T):
            e = int(ti[t])
            toks[e].append(t)
            slots_l[e].append(2 * t + k)
            cws[e].append(float(wi[t]))

    nblk = []
    perm = []
    sidx = []
    cw = []
    dump = 2 * T
    for e in range(E):
        n = len(toks[e])
        npd = math.ceil(n / BLK) * BLK if n > 0 else 0
        nblk.append(npd // BLK)
        perm.extend(toks[e])
        sidx.extend(slots_l[e])
        cw.extend(cws[e])
        for _ in range(npd - n):
            perm.append(0)
            sidx.append(dump)
            dump += 1
        cw.extend([0.0] * (npd - n))
    npad = len(perm)
    nslots = dump
    perm = np.asarray(perm, dtype=np.int64)

    # packed-transposed tokens, block-contiguous per partition:
    # xtp[p, b, dc, j] = x[perm[b*BLK + j], dc*128 + p]
    xr = _round_fp32r(x)
    nblk_tot = npad // BLK
    xtp = np.ascontiguousarray(
        xr[perm].reshape(nblk_tot, BLK, DCH, P).transpose(3, 0, 2, 1)
    )  # [P, nblk_tot, DCH, BLK]

    nsub = npad // P
    cw_a = np.asarray(cw, dtype=np.float32).reshape(nsub, P).T.copy()  # [P, nsub]
    sidx_a = np.asarray(sidx, dtype=np.int32).reshape(nsub, P).T.copy()  # [P, nsub]

    # per-core weights
    wst_all = []
    w2st_all = []
    gate = ws[:, :I, :]  # [E, I, D]
    up = ws[:, I:, :]
    for c in range(NCORES):
        lo, hi = c * ISH, (c + 1) * ISH
        # [E, DCH, P, 2*ISH]: [.., d-part, gate(ISH)||up(ISH)]
        g = gate[:, lo:hi, :].reshape(E, ISH, DCH, P).transpose(0, 2, 3, 1)
        u = up[:, lo:hi, :].reshape(E, ISH, DCH, P).transpose(0, 2, 3, 1)
        wst = np.concatenate([g, u], axis=3)
        wst_all.append(_round_fp32r(np.ascontiguousarray(wst)))
        # w2s[e] is [D, I]; w2sT slice = w2s[:, :, lo:hi].T -> [E, ISH, D] -> [E, ICH, P, D]
        w2t = w2s[:, :, lo:hi].transpose(0, 2, 1).reshape(E, ICH, P, D)
        w2st_all.append(_round_fp32r(np.ascontiguousarray(w2t)))

    shared = {"xtp": xtp, "cw": cw_a, "sidx": sidx_a}
    return nblk, npad, nslots, shared, wst_all, w2st_all


def kernel(hidden_states, router_w, ws, w2s):
    from concourse import bass_utils

    hs = np.asarray(hidden_states)
    B, S, _ = hs.shape
    nblk, npad, nslots, shared, wst_all, w2st_all = _prepare(hidden_states, router_w, ws, w2s)

    key = (tuple(nblk), npad, nslots)
    if key not in _CACHE:
        _CACHE[key] = _build_bass(nblk, npad, nslots)
    nc = _CACHE[key]

    in_maps = [
        {**shared, "wst": wst_all[c], "w2st": w2st_all[c]} for c in range(NCORES)
    ]
    res = bass_utils.run_bass_kernel_spmd(nc, in_maps, core_ids=list(range(NCORES)))
    out = np.concatenate([res.results[c]["out"] for c in range(NCORES)], axis=0)
    return out.reshape(B, S, D).astype(np.float32)


# revision 16
# speedup vs baseline: 14.5015x; 14.5015x over previous
"""DBRX MoE experts kernel for Trainium2 (8 NeuronCores).

Strategy:
  - Router (logits -> softmax -> top-2 -> renormalize) computed on host in numpy
    (0.01% of FLOPs); it determines the token->expert dispatch, i.e. the sharding.
  - Tensor-parallel over the FFN intermediate dim across 8 cores: core c owns
    I-slice [c*512:(c+1)*512) of every expert (ws rows for gate and up, w2s cols).
  - Top-2 sparsity: tokens are packed per expert (padded to 256-token blocks);
    each core runs gate/up matmuls (contraction D=2048), SwiGLU, down matmul
    (contraction I_shard=512), scales rows by combine weights, and writes the
    packed rows contiguously.
  - Matmuls run in fp32r (11-bit mantissa, full PE rate at free dim >= 256).
    All weight/activation inputs are pre-rounded to fp32r on host (bit-exact
    with the device rounding); the on-chip h = silu(gate)*up write rounds to
    fp32r for free via the DVE output dtype.
  - A ReduceScatter over the 8 cores sums the I-shard partials of the packed
    rows; core c returns packed rows [c*npad/8:(c+1)*npad/8). The host
    concatenates the shards and assembles out[t] = packed[pos0[t]] +
    packed[pos1[t]] (the two expert contributions, already weighted on device).
"""

import math

import numpy as np

T = 4096
D = 2048
E = 8
I = 4096
TOPK = 2
NCORES = 8
ISH = I // NCORES  # 512, per-core I shard
BLK = 256  # token block (matmul free dim for gate/up)
P = 128
DCH = D // P  # 16 d-chunks
ICH = ISH // P  # 4 i-chunks


def _round_fp32r(x: np.ndarray) -> np.ndarray:
    """Round-to-nearest-even to 11 explicit mantissa bits (device-verified bit-exact)."""
    b = np.ascontiguousarray(x, dtype=np.float32).view(np.uint32).astype(np.uint64)
    bias = ((b >> 12) & 1) + np.uint64(0x7FF)
    r = ((b + bias) >> 12 << 12).astype(np.uint32)
    return r.view(np.float32)


def _host_router(x, router_w):
    """Replicate reference routing in numpy (fp32)."""
    logits = (x.astype(np.float64) @ router_w.astype(np.float64).T).astype(np.float32)
    m = logits.max(axis=-1, keepdims=True)
    ex = np.exp((logits - m).astype(np.float32))
    probs = ex / ex.sum(axis=-1, keepdims=True)
    # top-2, ties to lower index (matches jax.lax.top_k)
    top1 = probs.argmax(axis=-1)
    p = probs.copy()
    p[np.arange(T), top1] = -1.0
    top2 = p.argmax(axis=-1)
    w1 = probs[np.arange(T), top1]
    w2 = probs[np.arange(T), top2]
    s = w1 + w2
    return top1.astype(np.int64), top2.astype(np.int64), (w1 / s).astype(np.float32), (w2 / s).astype(np.float32)


_CACHE: dict = {}


def _build_bass(nblk: list[int], npad: int):
    """Build the 8-core SPMD Bass program. nblk[e] = number of 256-token blocks
    for expert e; npad = total packed (padded) tokens."""
    import concourse.bacc as bacc
    import concourse.mybir as mybir
    import concourse.tile as tile

    f32 = mybir.dt.float32
    f32r = mybir.dt.float32r
    nsub = npad // P  # 128-row subblocks

    nc = bacc.Bacc("TRN2", target_bir_lowering=False)

    nblk_tot = npad // BLK
    xtp_d = nc.dram_tensor("xtp", [P, nblk_tot, DCH, BLK], f32r, kind="ExternalInput")
    wst_d = nc.dram_tensor("wst", [E, DCH, P, 2 * ISH], f32r, kind="ExternalInput")
    w2st_d = nc.dram_tensor("w2st", [E, ICH, P, D], f32r, kind="ExternalInput")
    cw_d = nc.dram_tensor("cw", [P, nsub], f32, kind="ExternalInput")
    out_d = nc.dram_tensor("out", [npad // NCORES, D], f32, kind="ExternalOutput")

    with tile.TileContext(nc) as tc:
        with (
            tc.tile_pool(name="dram", bufs=1, space="DRAM") as dram_pool,
            tc.tile_pool(name="wpool", bufs=18) as wpool,
            tc.tile_pool(name="w2pool", bufs=5) as w2pool,
            tc.tile_pool(name="xpool", bufs=2) as xpool,
            tc.tile_pool(name="spool", bufs=4) as spool,
            tc.tile_pool(name="hpool", bufs=2) as hpool,
            tc.tile_pool(name="opool", bufs=3) as opool,
            tc.tile_pool(name="const", bufs=1) as const_pool,
            tc.tile_pool(name="ph", bufs=4, space="PSUM") as ph_pool,
            tc.tile_pool(name="po", bufs=4, space="PSUM") as po_pool,
        ):
            packed = dram_pool.tile([npad, D], f32)
            rs_out = dram_pool.tile([npad // NCORES, D], f32)

            # combine weights, resident
            cw_sb = const_pool.tile([P, nsub], f32)
            nc.sync.dma_start(cw_sb[:], cw_d[:])

            gblk = 0
            for e in range(E):
                wst_tiles = []
                for dc in range(DCH):
                    wt = wpool.tile([P, 2 * ISH], f32r, tag="wst")
                    nc.sync.dma_start(wt[:], wst_d[e, dc])
                    wst_tiles.append(wt)
                w2_tiles = []
                for ic in range(ICH):
                    w2t = w2pool.tile([P, D], f32r, tag="w2st")
                    nc.sync.dma_start(w2t[:], w2st_d[e, ic])
                    w2_tiles.append(w2t)

                for _b in range(nblk[e]):
                    xt = xpool.tile([P, DCH, BLK], f32r, tag="xt")
                    nc.sync.dma_start(xt[:], xtp_d[:, gblk])

                    # phase 1: gate/up in ic-pairs; each accumulation group gets
                    # its own PSUM bank (start=True clears the whole bank)
                    hT = hpool.tile([P, ICH, BLK], f32r, tag="hT")
                    for half in range(ICH // 2):
                        phg = [
                            ph_pool.tile([P, BLK], f32, tag="ph", name=f"phg_{gblk}_{half}_{j}")
                            for j in range(2)
                        ]
                        phu = [
                            ph_pool.tile([P, BLK], f32, tag="ph", name=f"phu_{gblk}_{half}_{j}")
                            for j in range(2)
                        ]
                        for dc in range(DCH):
                            wt = wst_tiles[dc]
                            for j in range(2):
                                ic = half * 2 + j
                                nc.tensor.matmul(
                                    phg[j][:],
                                    wt[:, ic * P : (ic + 1) * P],
                                    xt[:, dc, :],
                                    start=(dc == 0),
                                    stop=(dc == DCH - 1),
                                )
                                nc.tensor.matmul(
                                    phu[j][:],
                                    wt[:, ISH + ic * P : ISH + (ic + 1) * P],
                                    xt[:, dc, :],
                                    start=(dc == 0),
                                    stop=(dc == DCH - 1),
                                )
                        for j in range(2):
                            ic = half * 2 + j
                            sg = spool.tile([P, BLK], f32, tag="sg")
                            nc.scalar.activation(
                                sg[:], phg[j][:], mybir.ActivationFunctionType.Silu
                            )
                            nc.vector.tensor_mul(hT[:, ic, :], sg[:], phu[j][:])

                    # phase 3: down proj per 128-token subblock
                    for s in range(BLK // P):
                        gsub = gblk * (BLK // P) + s
                        osb = opool.tile([P, D], f32, tag="osb")
                        for dt_i in range(D // 512):
                            po_t = po_pool.tile([P, 512], f32, tag="po")
                            for ic in range(ICH):
                                nc.tensor.matmul(
                                    po_t[:],
                                    hT[:, ic, s * P : (s + 1) * P],
                                    w2_tiles[ic][:, dt_i * 512 : (dt_i + 1) * 512],
                                    start=(ic == 0),
                                    stop=(ic == ICH - 1),
                                )
                            # evacuate + scale by combine weight (split ACT/DVE)
                            if dt_i < 2:
                                nc.scalar.activation(
                                    osb[:, dt_i * 512 : (dt_i + 1) * 512],
                                    po_t[:],
                                    mybir.ActivationFunctionType.Copy,
                                    scale=cw_sb[:, gsub : gsub + 1],
                                )
                            else:
                                nc.vector.tensor_scalar_mul(
                                    osb[:, dt_i * 512 : (dt_i + 1) * 512],
                                    po_t[:],
                                    cw_sb[:, gsub : gsub + 1],
                                )
                        nc.sync.dma_start(
                            packed[gsub * P : (gsub + 1) * P, :], osb[:]
                        )
                    gblk += 1

            nc.gpsimd.collective_compute(
                "ReduceScatter",
                mybir.AluOpType.add,
                replica_groups=[list(range(NCORES))],
                ins=[packed[:].opt()],
                outs=[rs_out[:].opt()],
            )
            nc.sync.dma_start(out_d[:], rs_out[:])

    nc.compile()
    return nc


def _prepare(hidden_states, router_w, ws, w2s):
    """Host-side routing, packing, transposes, fp32r rounding. Returns
    (nblk, npad, pos, shared inputs dict, per-core weight arrays)."""
    x = np.asarray(hidden_states, dtype=np.float32).reshape(T, D)
    router_w = np.asarray(router_w, dtype=np.float32)
    ws = np.asarray(ws, dtype=np.float32)
    w2s = np.asarray(w2s, dtype=np.float32)

    top1, top2, w1, w2 = _host_router(x, router_w)

    # per-expert token lists and weights
    toks: list[list[int]] = [[] for _ in range(E)]
    cws: list[list[float]] = [[] for _ in range(E)]
    for ti, wi in [(top1, w1), (top2, w2)]:
        for t in range(T):
            e = int(ti[t])
            toks[e].append(t)
            cws[e].append(float(wi[t]))

    nblk = []
    perm = []
    cw = []
    # pos[k, t] = packed position of token t's k-th expert contribution
    pos = np.zeros((TOPK, T), dtype=np.int64)
    seen = np.zeros(T, dtype=np.int64)
    for e in range(E):
        n = len(toks[e])
        npd = math.ceil(n / BLK) * BLK if n > 0 else 0
        nblk.append(npd // BLK)
        base = len(perm)
        for j, t in enumerate(toks[e]):
            pos[seen[t], t] = base + j
            seen[t] += 1
        perm.extend(toks[e])
        cw.extend(cws[e])
        perm.extend([0] * (npd - n))
        cw.extend([0.0] * (npd - n))
    npad = len(perm)
    perm = np.asarray(perm, dtype=np.int64)

    # packed-transposed tokens, block-contiguous per partition:
    # xtp[p, b, dc, j] = x[perm[b*BLK + j], dc*128 + p]
    xr = _round_fp32r(x)
    nblk_tot = npad // BLK
    xtp = np.ascontiguousarray(
        xr[perm].reshape(nblk_tot, BLK, DCH, P).transpose(3, 0, 2, 1)
    )  # [P, nblk_tot, DCH, BLK]

    nsub = npad // P
    cw_a = np.asarray(cw, dtype=np.float32).reshape(nsub, P).T.copy()  # [P, nsub]

    # per-core weights
    wst_all = []
    w2st_all = []
    gate = ws[:, :I, :]  # [E, I, D]
    up = ws[:, I:, :]
    for c in range(NCORES):
        lo, hi = c * ISH, (c + 1) * ISH
        # [E, DCH, P, 2*ISH]: [.., d-part, gate(ISH)||up(ISH)]
        g = gate[:, lo:hi, :].reshape(E, ISH, DCH, P).transpose(0, 2, 3, 1)
        u = up[:, lo:hi, :].reshape(E, ISH, DCH, P).transpose(0, 2, 3, 1)
        wst = np.concatenate([g, u], axis=3)
        wst_all.append(_round_fp32r(np.ascontiguousarray(wst)))
        # w2s[e] is [D, I]; w2sT slice = w2s[:, :, lo:hi].T -> [E, ISH, D] -> [E, ICH, P, D]
        w2t = w2s[:, :, lo:hi].transpose(0, 2, 1).reshape(E, ICH, P, D)
        w2st_all.append(_round_fp32r(np.ascontiguousarray(w2t)))

    shared = {"xtp": xtp, "cw": cw_a}
    return nblk, npad, pos, shared, wst_all, w2st_all


def kernel(hidden_states, router_w, ws, w2s):
    from concourse import bass_utils

    hs = np.asarray(hidden_states)
    B, S, _ = hs.shape
    nblk, npad, pos, shared, wst_all, w2st_all = _prepare(hidden_states, router_w, ws, w2s)

    key = (tuple(nblk), npad)
    if key not in _CACHE:
        _CACHE[key] = _build_bass(nblk, npad)
    nc = _CACHE[key]

    in_maps = [
        {**shared, "wst": wst_all[c], "w2st": w2st_all[c]} for c in range(NCORES)
    ]
    res = bass_utils.run_bass_kernel_spmd(nc, in_maps, core_ids=list(range(NCORES)))
    packed = np.concatenate([res.results[c]["out"] for c in range(NCORES)], axis=0)
    out = packed[pos[0]] + packed[pos[1]]  # the two (device-weighted) expert contributions
    return out.reshape(B, S, D).astype(np.float32)


# revision 18
# speedup vs baseline: 17.3373x; 1.1956x over previous
"""DBRX MoE experts kernel for Trainium2 (8 NeuronCores).

Strategy:
  - Router (logits -> softmax -> top-2 -> renormalize) computed on host in numpy
    (0.01% of FLOPs); it determines the token->expert dispatch, i.e. the sharding.
  - Tensor-parallel over the FFN intermediate dim across 8 cores: core c owns
    I-slice [c*512:(c+1)*512) of every expert (ws rows for gate and up, w2s cols).
  - Top-2 sparsity: tokens are packed per expert (padded to 256-token blocks);
    each core runs gate/up matmuls (contraction D=2048), SwiGLU, down matmul
    (contraction I_shard=512), scales rows by combine weights, and writes the
    packed rows contiguously.
  - Matmuls run in fp32r (11-bit mantissa, full PE rate at free dim >= 256).
    All weight/activation inputs are pre-rounded to fp32r on host (bit-exact
    with the device rounding); the on-chip h = silu(gate)*up write rounds to
    fp32r for free via the DVE output dtype.
  - A ReduceScatter over the 8 cores sums the I-shard partials of the packed
    rows; core c returns packed rows [c*npad/8:(c+1)*npad/8). The host
    concatenates the shards and assembles out[t] = packed[pos0[t]] +
    packed[pos1[t]] (the two expert contributions, already weighted on device).
"""

import math

import numpy as np

T = 4096
D = 2048
E = 8
I = 4096
TOPK = 2
NCORES = 8
ISH = I // NCORES  # 512, per-core I shard
BLK = 256  # token block (matmul free dim for gate/up)
P = 128
DCH = D // P  # 16 d-chunks
ICH = ISH // P  # 4 i-chunks


def _round_fp32r(x: np.ndarray) -> np.ndarray:
    """Round-to-nearest-even to 11 explicit mantissa bits (device-verified bit-exact)."""
    b = np.ascontiguousarray(x, dtype=np.float32).view(np.uint32).astype(np.uint64)
    bias = ((b >> 12) & 1) + np.uint64(0x7FF)
    r = ((b + bias) >> 12 << 12).astype(np.uint32)
    return r.view(np.float32)


def _host_router(x, router_w):
    """Replicate reference routing in numpy (fp32)."""
    logits = (x.astype(np.float64) @ router_w.astype(np.float64).T).astype(np.float32)
    m = logits.max(axis=-1, keepdims=True)
    ex = np.exp((logits - m).astype(np.float32))
    probs = ex / ex.sum(axis=-1, keepdims=True)
    # top-2, ties to lower index (matches jax.lax.top_k)
    top1 = probs.argmax(axis=-1)
    p = probs.copy()
    p[np.arange(T), top1] = -1.0
    top2 = p.argmax(axis=-1)
    w1 = probs[np.arange(T), top1]
    w2 = probs[np.arange(T), top2]
    s = w1 + w2
    return top1.astype(np.int64), top2.astype(np.int64), (w1 / s).astype(np.float32), (w2 / s).astype(np.float32)


_CACHE: dict = {}


def _build_bass(nblk: list[int], npad: int):
    """Build the 8-core SPMD Bass program. nblk[e] = number of 256-token blocks
    for expert e; npad = total packed (padded) tokens."""
    import concourse.bacc as bacc
    import concourse.mybir as mybir
    import concourse.tile as tile

    f32 = mybir.dt.float32
    f32r = mybir.dt.float32r
    nsub = npad // P  # 128-row subblocks

    nc = bacc.Bacc("TRN2", target_bir_lowering=False)

    nblk_tot = npad // BLK
    xtp_d = nc.dram_tensor("xtp", [P, nblk_tot, DCH, BLK], f32r, kind="ExternalInput")
    wst_d = nc.dram_tensor("wst", [E, DCH, P, 2 * ISH], f32r, kind="ExternalInput")
    w2st_d = nc.dram_tensor("w2st", [E, ICH, P, D], f32r, kind="ExternalInput")
    cw_d = nc.dram_tensor("cw", [P, nsub], f32, kind="ExternalInput")
    out_d = nc.dram_tensor("out", [npad // NCORES, D], f32, kind="ExternalOutput")

    with tile.TileContext(nc) as tc:
        with (
            tc.tile_pool(name="dram", bufs=1, space="DRAM") as dram_pool,
            tc.tile_pool(name="wpool", bufs=18) as wpool,
            tc.tile_pool(name="w2pool", bufs=5) as w2pool,
            tc.tile_pool(name="xpool", bufs=2) as xpool,
            tc.tile_pool(name="spool", bufs=4) as spool,
            tc.tile_pool(name="hpool", bufs=2) as hpool,
            tc.tile_pool(name="opool", bufs=3) as opool,
            tc.tile_pool(name="const", bufs=1) as const_pool,
            tc.tile_pool(name="ph", bufs=4, space="PSUM") as ph_pool,
            tc.tile_pool(name="po", bufs=4, space="PSUM") as po_pool,
        ):
            packed = dram_pool.tile([npad, D], f32)
            rs_out = dram_pool.tile([npad // NCORES, D], f32)

            # combine weights, resident
            cw_sb = const_pool.tile([P, nsub], f32)
            nc.sync.dma_start(cw_sb[:], cw_d[:])

            gblk = 0
            for e in range(E):
                wst_tiles = []
                for dc in range(DCH):
                    wt = wpool.tile([P, 2 * ISH], f32r, tag="wst")
                    nc.sync.dma_start(wt[:], wst_d[e, dc])
                    wst_tiles.append(wt)
                w2_tiles = []
                for ic in range(ICH):
                    w2t = w2pool.tile([P, D], f32r, tag="w2st")
                    nc.sync.dma_start(w2t[:], w2st_d[e, ic])
                    w2_tiles.append(w2t)

                for _b in range(nblk[e]):
                    xt = xpool.tile([P, DCH, BLK], f32r, tag="xt")
                    nc.sync.dma_start(xt[:], xtp_d[:, gblk])

                    # phase 1: gate/up in ic-pairs; each accumulation group gets
                    # its own PSUM bank (start=True clears the whole bank)
                    hT = hpool.tile([P, ICH, BLK], f32r, tag="hT")
                    for half in range(ICH // 2):
                        phg = [
                            ph_pool.tile([P, BLK], f32, tag="ph", name=f"phg_{gblk}_{half}_{j}")
                            for j in range(2)
                        ]
                        phu = [
                            ph_pool.tile([P, BLK], f32, tag="ph", name=f"phu_{gblk}_{half}_{j}")
                            for j in range(2)
                        ]
                        for dc in range(DCH):
                            wt = wst_tiles[dc]
                            for j in range(2):
                                ic = half * 2 + j
                                nc.tensor.matmul(
                                    phg[j][:],
                                    wt[:, ic * P : (ic + 1) * P],
                                    xt[:, dc, :],
                                    start=(dc == 0),
                                    stop=(dc == DCH - 1),
                                )
                                nc.tensor.matmul(
                                    phu[j][:],
                                    wt[:, ISH + ic * P : ISH + (ic + 1) * P],
                                    xt[:, dc, :],
                                    start=(dc == 0),
                                    stop=(dc == DCH - 1),
                                )
                        for j in range(2):
                            ic = half * 2 + j
                            sg = spool.tile([P, BLK], f32, tag="sg")
                            nc.scalar.activation(
                                sg[:], phg[j][:], mybir.ActivationFunctionType.Silu
                            )
                            nc.vector.tensor_mul(hT[:, ic, :], sg[:], phu[j][:])

                    # phase 3: down proj per 128-token subblock
                    for s in range(BLK // P):
                        gsub = gblk * (BLK // P) + s
                        osb = opool.tile([P, D], f32, tag="osb")
                        for dt_i in range(D // 512):
                            po_t = po_pool.tile([P, 512], f32, tag="po")
                            for ic in range(ICH):
                                nc.tensor.matmul(
                                    po_t[:],
                                    hT[:, ic, s * P : (s + 1) * P],
                                    w2_tiles[ic][:, dt_i * 512 : (dt_i + 1) * 512],
                                    start=(ic == 0),
                                    stop=(ic == ICH - 1),
                                )
                            # evacuate + scale by combine weight (split ACT/DVE)
                            if dt_i < 2:
                                nc.scalar.activation(
                                    osb[:, dt_i * 512 : (dt_i + 1) * 512],
                                    po_t[:],
                                    mybir.ActivationFunctionType.Copy,
                                    scale=cw_sb[:, gsub : gsub + 1],
                                )
                            else:
                                nc.vector.tensor_scalar_mul(
                                    osb[:, dt_i * 512 : (dt_i + 1) * 512],
                                    po_t[:],
                                    cw_sb[:, gsub : gsub + 1],
                                )
                        nc.sync.dma_start(
                            packed[gsub * P : (gsub + 1) * P, :], osb[:]
                        )
                    gblk += 1

                # expert e's packed rows are final on every core here; reduce-
                # scatter them now so the collective overlaps the next expert
                base = (gblk - nblk[e]) * BLK
                sz = nblk[e] * BLK
                nc.gpsimd.collective_compute(
                    "ReduceScatter",
                    mybir.AluOpType.add,
                    replica_groups=[list(range(NCORES))],
                    ins=[packed[base : base + sz].opt()],
                    outs=[rs_out[base // NCORES : (base + sz) // NCORES].opt()],
                )
            nc.sync.dma_start(out_d[:], rs_out[:])

    nc.compile()
    return nc


def _prepare(hidden_states, router_w, ws, w2s):
    """Host-side routing, packing, transposes, fp32r rounding. Returns
    (nblk, npad, pos, shared inputs dict, per-core weight arrays)."""
    x = np.asarray(hidden_states, dtype=np.float32).reshape(T, D)
    router_w = np.asarray(router_w, dtype=np.float32)
    ws = np.asarray(ws, dtype=np.float32)
    w2s = np.asarray(w2s, dtype=np.float32)

    top1, top2, w1, w2 = _host_router(x, router_w)

    # per-expert token lists and weights
    toks: list[list[int]] = [[] for _ in range(E)]
    cws: list[list[float]] = [[] for _ in range(E)]
    for ti, wi in [(top1, w1), (top2, w2)]:
        for t in range(T):
            e = int(ti[t])
            toks[e].append(t)
            cws[e].append(float(wi[t]))

    nblk = []
    perm = []
    cw = []
    # pos[k, t] = packed position of token t's k-th expert contribution
    pos = np.zeros((TOPK, T), dtype=np.int64)
    seen = np.zeros(T, dtype=np.int64)
    for e in range(E):
        n = len(toks[e])
        npd = math.ceil(n / BLK) * BLK if n > 0 else 0
        nblk.append(npd // BLK)
        base = len(perm)
        for j, t in enumerate(toks[e]):
            pos[seen[t], t] = base + j
            seen[t] += 1
        perm.extend(toks[e])
        cw.extend(cws[e])
        perm.extend([0] * (npd - n))
        cw.extend([0.0] * (npd - n))
    npad = len(perm)
    perm = np.asarray(perm, dtype=np.int64)

    # packed-transposed tokens, block-contiguous per partition:
    # xtp[p, b, dc, j] = x[perm[b*BLK + j], dc*128 + p]
    xr = _round_fp32r(x)
    nblk_tot = npad // BLK
    xtp = np.ascontiguousarray(
        xr[perm].reshape(nblk_tot, BLK, DCH, P).transpose(3, 0, 2, 1)
    )  # [P, nblk_tot, DCH, BLK]

    nsub = npad // P
    cw_a = np.asarray(cw, dtype=np.float32).reshape(nsub, P).T.copy()  # [P, nsub]

    # per-core weights
    wst_all = []
    w2st_all = []
    gate = ws[:, :I, :]  # [E, I, D]
    up = ws[:, I:, :]
    for c in range(NCORES):
        lo, hi = c * ISH, (c + 1) * ISH
        # [E, DCH, P, 2*ISH]: [.., d-part, gate(ISH)||up(ISH)]
        g = gate[:, lo:hi, :].reshape(E, ISH, DCH, P).transpose(0, 2, 3, 1)
        u = up[:, lo:hi, :].reshape(E, ISH, DCH, P).transpose(0, 2, 3, 1)
        wst = np.concatenate([g, u], axis=3)
        wst_all.append(_round_fp32r(np.ascontiguousarray(wst)))
        # w2s[e] is [D, I]; w2sT slice = w2s[:, :, lo:hi].T -> [E, ISH, D] -> [E, ICH, P, D]
        w2t = w2s[:, :, lo:hi].transpose(0, 2, 1).reshape(E, ICH, P, D)
        w2st_all.append(_round_fp32r(np.ascontiguousarray(w2t)))

    shared = {"xtp": xtp, "cw": cw_a}
    return nblk, npad, pos, shared, wst_all, w2st_all


def kernel(hidden_states, router_w, ws, w2s):
    from concourse import bass_utils

    hs = np.asarray(hidden_states)
    B, S, _ = hs.shape
    nblk, npad, pos, shared, wst_all, w2st_all = _prepare(hidden_states, router_w, ws, w2s)

    key = (tuple(nblk), npad)
    if key not in _CACHE:
        _CACHE[key] = _build_bass(nblk, npad)
    nc = _CACHE[key]

    in_maps = [
        {**shared, "wst": wst_all[c], "w2st": w2st_all[c]} for c in range(NCORES)
    ]
    res = bass_utils.run_bass_kernel_spmd(nc, in_maps, core_ids=list(range(NCORES)))
    # per-expert chunked RS: within each expert's row range, core c holds the
    # c-th eighth; reassemble the full packed array
    npad_total = sum(nblk) * BLK
    packed = np.empty((npad_total, D), dtype=np.float32)
    base = 0
    for e in range(E):
        sz = nblk[e] * BLK
        sz8 = sz // NCORES
        for c in range(NCORES):
            packed[base + c * sz8 : base + (c + 1) * sz8] = res.results[c]["out"][
                base // NCORES : base // NCORES + sz8
            ]
        base += sz
    out = packed[pos[0]] + packed[pos[1]]  # the two (device-weighted) expert contributions
    return out.reshape(B, S, D).astype(np.float32)


# revision 20
# speedup vs baseline: 18.2090x; 1.0503x over previous
"""DBRX MoE experts kernel for Trainium2 (8 NeuronCores).

Strategy:
  - Router (logits -> softmax -> top-2 -> renormalize) computed on host in numpy
    (0.01% of FLOPs); it determines the token->expert dispatch, i.e. the sharding.
  - Tensor-parallel over the FFN intermediate dim across 8 cores: core c owns
    I-slice [c*512:(c+1)*512) of every expert (ws rows for gate and up, w2s cols).
  - Top-2 sparsity: tokens are packed per expert (padded to 256-token blocks);
    each core runs gate/up matmuls (contraction D=2048), SwiGLU, down matmul
    (contraction I_shard=512), scales rows by combine weights, and writes the
    packed rows contiguously.
  - Matmuls run in fp32r (11-bit mantissa, full PE rate at free dim >= 256).
    All weight/activation inputs are pre-rounded to fp32r on host (bit-exact
    with the device rounding); the on-chip h = silu(gate)*up write rounds to
    fp32r for free via the DVE output dtype.
  - A ReduceScatter over the 8 cores sums the I-shard partials of the packed
    rows; core c returns packed rows [c*npad/8:(c+1)*npad/8). The host
    concatenates the shards and assembles out[t] = packed[pos0[t]] +
    packed[pos1[t]] (the two expert contributions, already weighted on device).
"""

import math

import numpy as np

T = 4096
D = 2048
E = 8
I = 4096
TOPK = 2
NCORES = 8
ISH = I // NCORES  # 512, per-core I shard
BLK = 256  # token block (matmul free dim for gate/up)
P = 128
DCH = D // P  # 16 d-chunks
ICH = ISH // P  # 4 i-chunks


def _round_fp32r(x: np.ndarray) -> np.ndarray:
    """Round-to-nearest-even to 11 explicit mantissa bits (device-verified bit-exact)."""
    b = np.ascontiguousarray(x, dtype=np.float32).view(np.uint32).astype(np.uint64)
    bias = ((b >> 12) & 1) + np.uint64(0x7FF)
    r = ((b + bias) >> 12 << 12).astype(np.uint32)
    return r.view(np.float32)


def _host_router(x, router_w):
    """Replicate reference routing in numpy (fp32)."""
    logits = (x.astype(np.float64) @ router_w.astype(np.float64).T).astype(np.float32)
    m = logits.max(axis=-1, keepdims=True)
    ex = np.exp((logits - m).astype(np.float32))
    probs = ex / ex.sum(axis=-1, keepdims=True)
    # top-2, ties to lower index (matches jax.lax.top_k)
    top1 = probs.argmax(axis=-1)
    p = probs.copy()
    p[np.arange(T), top1] = -1.0
    top2 = p.argmax(axis=-1)
    w1 = probs[np.arange(T), top1]
    w2 = probs[np.arange(T), top2]
    s = w1 + w2
    return top1.astype(np.int64), top2.astype(np.int64), (w1 / s).astype(np.float32), (w2 / s).astype(np.float32)


_CACHE: dict = {}


def _build_bass(nblk: list[int], npad: int):
    """Build the 8-core SPMD Bass program. nblk[e] = number of 256-token blocks
    for expert e; npad = total packed (padded) tokens."""
    import concourse.bacc as bacc
    import concourse.mybir as mybir
    import concourse.tile as tile

    f32 = mybir.dt.float32
    f32r = mybir.dt.float32r
    nsub = npad // P  # 128-row subblocks

    nc = bacc.Bacc("TRN2", target_bir_lowering=False)

    nblk_tot = npad // BLK
    xtp_d = nc.dram_tensor("xtp", [P, nblk_tot, DCH, BLK], f32r, kind="ExternalInput")
    wst_d = nc.dram_tensor("wst", [E, DCH, P, 2 * ISH], f32r, kind="ExternalInput")
    w2st_d = nc.dram_tensor("w2st", [E, ICH, P, D], f32r, kind="ExternalInput")
    cw_d = nc.dram_tensor("cw", [P, nsub], f32, kind="ExternalInput")
    out_d = nc.dram_tensor("out", [npad // NCORES, D], f32, kind="ExternalOutput")

    with tile.TileContext(nc) as tc:
        with (
            tc.tile_pool(name="dram", bufs=1, space="DRAM") as dram_pool,
            tc.tile_pool(name="wpool", bufs=20) as wpool,
            tc.tile_pool(name="w2pool", bufs=5) as w2pool,
            tc.tile_pool(name="xpool", bufs=2) as xpool,
            tc.tile_pool(name="spool", bufs=4) as spool,
            tc.tile_pool(name="hpool", bufs=2) as hpool,
            tc.tile_pool(name="opool", bufs=3) as opool,
            tc.tile_pool(name="const", bufs=1) as const_pool,
            tc.tile_pool(name="ph", bufs=4, space="PSUM") as ph_pool,
            tc.tile_pool(name="po", bufs=4, space="PSUM") as po_pool,
        ):
            packed = dram_pool.tile([npad, D], f32)
            rs_out = dram_pool.tile([npad // NCORES, D], f32)

            # combine weights, resident
            cw_sb = const_pool.tile([P, nsub], f32)
            nc.sync.dma_start(cw_sb[:], cw_d[:])

            # first token block issued before any weights so the first matmul's
            # deps (xt0 + wst tile 0) are at the head of the DMA queue
            xt0 = xpool.tile([P, DCH, BLK], f32r, tag="xt")
            nc.sync.dma_start(xt0[:], xtp_d[:, 0])

            gblk = 0
            for e in range(E):
                wst_tiles = []
                for dc in range(DCH):
                    wt = wpool.tile([P, 2 * ISH], f32r, tag="wst")
                    nc.sync.dma_start(wt[:], wst_d[e, dc])
                    wst_tiles.append(wt)
                w2_tiles = []
                for ic in range(ICH):
                    w2t = w2pool.tile([P, D], f32r, tag="w2st")
                    nc.sync.dma_start(w2t[:], w2st_d[e, ic])
                    w2_tiles.append(w2t)

                for _b in range(nblk[e]):
                    if gblk == 0:
                        xt = xt0
                    else:
                        xt = xpool.tile([P, DCH, BLK], f32r, tag="xt")
                        nc.sync.dma_start(xt[:], xtp_d[:, gblk])

                    # phase 1: gate/up in ic-pairs; each accumulation group gets
                    # its own PSUM bank (start=True clears the whole bank)
                    hT = hpool.tile([P, ICH, BLK], f32r, tag="hT")
                    for half in range(ICH // 2):
                        phg = [
                            ph_pool.tile([P, BLK], f32, tag="ph", name=f"phg_{gblk}_{half}_{j}")
                            for j in range(2)
                        ]
                        phu = [
                            ph_pool.tile([P, BLK], f32, tag="ph", name=f"phu_{gblk}_{half}_{j}")
                            for j in range(2)
                        ]
                        for dc in range(DCH):
                            wt = wst_tiles[dc]
                            for j in range(2):
                                ic = half * 2 + j
                                nc.tensor.matmul(
                                    phg[j][:],
                                    wt[:, ic * P : (ic + 1) * P],
                                    xt[:, dc, :],
                                    start=(dc == 0),
                                    stop=(dc == DCH - 1),
                                )
                                nc.tensor.matmul(
                                    phu[j][:],
                                    wt[:, ISH + ic * P : ISH + (ic + 1) * P],
                                    xt[:, dc, :],
                                    start=(dc == 0),
                                    stop=(dc == DCH - 1),
                                )
                        for j in range(2):
                            ic = half * 2 + j
                            sg = spool.tile([P, BLK], f32, tag="sg")
                            nc.scalar.activation(
                                sg[:], phg[j][:], mybir.ActivationFunctionType.Silu
                            )
                            nc.vector.tensor_mul(hT[:, ic, :], sg[:], phu[j][:])

                    # phase 3: down proj per 128-token subblock
                    for s in range(BLK // P):
                        gsub = gblk * (BLK // P) + s
                        osb = opool.tile([P, D], f32, tag="osb")
                        for dt_i in range(D // 512):
                            po_t = po_pool.tile([P, 512], f32, tag="po")
                            for ic in range(ICH):
                                nc.tensor.matmul(
                                    po_t[:],
                                    hT[:, ic, s * P : (s + 1) * P],
                                    w2_tiles[ic][:, dt_i * 512 : (dt_i + 1) * 512],
                                    start=(ic == 0),
                                    stop=(ic == ICH - 1),
                                )
                            # evacuate + scale by combine weight (split ACT/DVE)
                            if dt_i < 2:
                                nc.scalar.activation(
                                    osb[:, dt_i * 512 : (dt_i + 1) * 512],
                                    po_t[:],
                                    mybir.ActivationFunctionType.Copy,
                                    scale=cw_sb[:, gsub : gsub + 1],
                                )
                            else:
                                nc.vector.tensor_scalar_mul(
                                    osb[:, dt_i * 512 : (dt_i + 1) * 512],
                                    po_t[:],
                                    cw_sb[:, gsub : gsub + 1],
                                )
                        nc.sync.dma_start(
                            packed[gsub * P : (gsub + 1) * P, :], osb[:]
                        )
                    gblk += 1

                # expert e's packed rows are final on every core here; reduce-
                # scatter them now so the collective overlaps the next expert
                base = (gblk - nblk[e]) * BLK
                sz = nblk[e] * BLK
                nc.gpsimd.collective_compute(
                    "ReduceScatter",
                    mybir.AluOpType.add,
                    replica_groups=[list(range(NCORES))],
                    ins=[packed[base : base + sz].opt()],
                    outs=[rs_out[base // NCORES : (base + sz) // NCORES].opt()],
                )
            nc.sync.dma_start(out_d[:], rs_out[:])

    nc.compile()
    return nc


def _prepare(hidden_states, router_w, ws, w2s):
    """Host-side routing, packing, transposes, fp32r rounding. Returns
    (nblk, npad, pos, shared inputs dict, per-core weight arrays)."""
    x = np.asarray(hidden_states, dtype=np.float32).reshape(T, D)
    router_w = np.asarray(router_w, dtype=np.float32)
    ws = np.asarray(ws, dtype=np.float32)
    w2s = np.asarray(w2s, dtype=np.float32)

    top1, top2, w1, w2 = _host_router(x, router_w)

    # per-expert token lists and weights
    toks: list[list[int]] = [[] for _ in range(E)]
    cws: list[list[float]] = [[] for _ in range(E)]
    for ti, wi in [(top1, w1), (top2, w2)]:
        for t in range(T):
            e = int(ti[t])
            toks[e].append(t)
            cws[e].append(float(wi[t]))

    nblk = []
    perm = []
    cw = []
    # pos[k, t] = packed position of token t's k-th expert contribution
    pos = np.zeros((TOPK, T), dtype=np.int64)
    seen = np.zeros(T, dtype=np.int64)
    for e in range(E):
        n = len(toks[e])
        npd = math.ceil(n / BLK) * BLK if n > 0 else 0
        nblk.append(npd // BLK)
        base = len(perm)
        for j, t in enumerate(toks[e]):
            pos[seen[t], t] = base + j
            seen[t] += 1
        perm.extend(toks[e])
        cw.extend(cws[e])
        perm.extend([0] * (npd - n))
        cw.extend([0.0] * (npd - n))
    npad = len(perm)
    perm = np.asarray(perm, dtype=np.int64)

    # packed-transposed tokens, block-contiguous per partition:
    # xtp[p, b, dc, j] = x[perm[b*BLK + j], dc*128 + p]
    xr = _round_fp32r(x)
    nblk_tot = npad // BLK
    xtp = np.ascontiguousarray(
        xr[perm].reshape(nblk_tot, BLK, DCH, P).transpose(3, 0, 2, 1)
    )  # [P, nblk_tot, DCH, BLK]

    nsub = npad // P
    cw_a = np.asarray(cw, dtype=np.float32).reshape(nsub, P).T.copy()  # [P, nsub]

    # per-core weights
    wst_all = []
    w2st_all = []
    gate = ws[:, :I, :]  # [E, I, D]
    up = ws[:, I:, :]
    for c in range(NCORES):
        lo, hi = c * ISH, (c + 1) * ISH
        # [E, DCH, P, 2*ISH]: [.., d-part, gate(ISH)||up(ISH)]
        g = gate[:, lo:hi, :].reshape(E, ISH, DCH, P).transpose(0, 2, 3, 1)
        u = up[:, lo:hi, :].reshape(E, ISH, DCH, P).transpose(0, 2, 3, 1)
        wst = np.concatenate([g, u], axis=3)
        wst_all.append(_round_fp32r(np.ascontiguousarray(wst)))
        # w2s[e] is [D, I]; w2sT slice = w2s[:, :, lo:hi].T -> [E, ISH, D] -> [E, ICH, P, D]
        w2t = w2s[:, :, lo:hi].transpose(0, 2, 1).reshape(E, ICH, P, D)
        w2st_all.append(_round_fp32r(np.ascontiguousarray(w2t)))

    shared = {"xtp": xtp, "cw": cw_a}
    return nblk, npad, pos, shared, wst_all, w2st_all


def kernel(hidden_states, router_w, ws, w2s):
    from concourse import bass_utils

    hs = np.asarray(hidden_states)
    B, S, _ = hs.shape
    nblk, npad, pos, shared, wst_all, w2st_all = _prepare(hidden_states, router_w, ws, w2s)

    key = (tuple(nblk), npad)
    if key not in _CACHE:
        _CACHE[key] = _build_bass(nblk, npad)
    nc = _CACHE[key]

    in_maps = [
        {**shared, "wst": wst_all[c], "w2st": w2st_all[c]} for c in range(NCORES)
    ]
    res = bass_utils.run_bass_kernel_spmd(nc, in_maps, core_ids=list(range(NCORES)))
    # per-expert chunked RS: within each expert's row range, core c holds the
    # c-th eighth; reassemble the full packed array
    npad_total = sum(nblk) * BLK
    packed = np.empty((npad_total, D), dtype=np.float32)
    base = 0
    for e in range(E):
        sz = nblk[e] * BLK
        sz8 = sz // NCORES
        for c in range(NCORES):
            packed[base + c * sz8 : base + (c + 1) * sz8] = res.results[c]["out"][
                base // NCORES : base // NCORES + sz8
            ]
        base += sz
    out = packed[pos[0]] + packed[pos[1]]  # the two (device-weighted) expert contributions
    return out.reshape(B, S, D).astype(np.float32)
